# revision 15
# baseline (speedup 1.0000x reference)
"""Gaussian splatting tile rasterizer for 8 TRN2 NeuronCores.

Strategy (hardcoded for the nn_GaussRenderer problem, H=W=128, TILE=32,
N=12000):
  - Host (numpy, O(N)): per-gaussian prep — conic / radii / rect exactly as
    the reference computes them in fp32, one global stable depth argsort,
    per-tile cull + compaction (per-tile survivor lists stay in global depth
    order, so per-tile composite order matches the reference, since culled
    gaussians contribute alpha=0).  The gaussian exponent
        q(p,j) = -0.5 dx^T conic dx + ln(opacity)
    is bilinear in per-pixel features f(p)=[px^2,py^2,px*py,px,py,1]
    (tile-centered coords) and per-gaussian coeffs g(j), so the device
    computes it as a K=6 matmul.
  - Device (SPMD, 2 image tiles per core, 8 pixel-blocks of 128 pixels per
    tile, G gaussian slots padded to PADG):
      PE:  q[128pix, PADG] = f_blk^T @ g          (K=6 matmul)
      ACT: e = exp(q)
      DVE: u = 1 - min(e, 0.99)                   (2x tensor_scalar ops)
      DVE: P = inclusive cumprod of u             (tensor_tensor_scan)
      PE:  P^T per 128-block                      (transpose via identity)
      ACT: copy P^T PSUM->SBUF
      PE:  img^T[5, pix] += dF_blk^T @ P^T_blk    (Abel summation: with
            w_j = P_{j-1}-P_j, sum_j w_j F_j = F_0 + sum_j P_j (F_{j+1}-F_j))
      ACT: img + per-tile bias (F_0 + white background fold) -> SBUF -> DMA
  - Host: scatter per-core [2, 5, 1024] outputs into img_c/img_d/img_a.
"""

import numpy as np

H = 128
W = 128
TILE = 32
N_CORES = 8
TILES_PER_CORE = 2
PADG = 1536           # max per-tile survivor count is ~1302 for the ref seed
NBLK = PADG // 128    # 12 gaussian blocks of 128
NEG = np.float32(-1e30)

_CACHE = {}


# ----------------------------------------------------------------------------
# host-side per-gaussian prep (exact fp32 replication of the reference)
# ----------------------------------------------------------------------------

def _prep(means2D, cov2d, color, opacity, depths):
    means2D = np.asarray(means2D, np.float32)
    cov2d = np.asarray(cov2d, np.float32)
    color = np.asarray(color, np.float32)
    opacity = np.asarray(opacity, np.float32)
    depths = np.asarray(depths, np.float32)

    c00 = cov2d[:, 0, 0]
    c01 = cov2d[:, 0, 1]
    c10 = cov2d[:, 1, 0]
    c11 = cov2d[:, 1, 1]
    det = c00 * c11 - c01 * c10
    mid = np.float32(0.5) * (c00 + c11)
    s = np.sqrt(np.maximum(mid * mid - det, np.float32(0.1)))
    radii = np.float32(3.0) * np.ceil(np.sqrt(np.maximum(mid + s, mid - s)))
    rmin = np.clip(means2D - radii[:, None], 0.0, W - 1.0).astype(np.float32)
    rmax = np.clip(means2D + radii[:, None], 0.0, W - 1.0).astype(np.float32)
    inv_det = np.float32(1.0) / det
    a_ = c11 * inv_det
    c_ = c00 * inv_det
    b_ = (-c01 - c10) * inv_det   # conic01 + conic10

    order = np.argsort(depths, kind="stable")
    lnop = np.log(np.maximum(opacity[:, 0], np.float32(1e-38))).astype(np.float32)

    # per-tile survivor counts -> per-slot padded gaussian counts
    sels = []
    for h in range(0, H, TILE):
        for w in range(0, W, TILE):
            tlx = np.maximum(rmin[:, 0], w)
            tly = np.maximum(rmin[:, 1], h)
            brx = np.minimum(rmax[:, 0], w + TILE - 1.0)
            bry = np.minimum(rmax[:, 1], h + TILE - 1.0)
            mask = (brx > tlx) & (bry > tly)
            sels.append(order[mask[order]])
    counts = [len(s) for s in sels]
    padg = []
    for slot in range(TILES_PER_CORE):
        m = max(counts[slot::TILES_PER_CORE])
        padg.append(max(128, -(-m // 128) * 128))
    pmax = max(padg)
    if pmax > 4096:
        return None  # fallback path

    feat = np.zeros((16, 6, 1024), np.float32)
    gco = np.zeros((16, 6, pmax), np.float32)
    gco[:, 5, :] = NEG                       # padding slots -> q = -1e30
    dft = np.zeros((16, pmax, 5), np.float32)
    bias = np.zeros((16, 5), np.float32)
    bias[:, 0:3] = 1.0                       # white background fold

    t = 0
    for h in range(0, H, TILE):
        for w in range(0, W, TILE):
            sel = sels[t]
            k = len(sel)

            cx = np.float32(w + 15.5)
            cy = np.float32(h + 15.5)
            mx = (means2D[sel, 0] - cx).astype(np.float32)
            my = (means2D[sel, 1] - cy).astype(np.float32)
            aa = a_[sel]
            cc = c_[sel]
            bb = b_[sel]
            gco[t, 0, :k] = -aa / 2
            gco[t, 1, :k] = -cc / 2
            gco[t, 2, :k] = -bb / 2
            gco[t, 3, :k] = aa * mx + bb * my / 2
            gco[t, 4, :k] = cc * my + bb * mx / 2
            gco[t, 5, :k] = (
                np.float32(-0.5) * (aa * mx * mx + cc * my * my + bb * mx * my)
                + lnop[sel]
            )

            ys, xs = np.meshgrid(np.arange(TILE), np.arange(TILE), indexing="ij")
            px = (xs.ravel() + w - cx).astype(np.float32)
            py = (ys.ravel() + h - cy).astype(np.float32)
            feat[t, 0] = px * px
            feat[t, 1] = py * py
            feat[t, 2] = px * py
            feat[t, 3] = px
            feat[t, 4] = py
            feat[t, 5] = 1.0

            F = np.zeros((k + 1, 5), np.float32)
            F[:k, 0] = color[sel, 0] - 1.0
            F[:k, 1] = color[sel, 1] - 1.0
            F[:k, 2] = color[sel, 2] - 1.0
            F[:k, 3] = depths[sel]
            F[:k, 4] = 1.0
            dft[t, :k] = F[1:] - F[:-1]
            if k > 0:
                bias[t] += F[0]
            t += 1
    return feat, gco, dft, bias, padg


def _render_numpy(means2D, cov2d, color, opacity, depths):
    """Exact fallback (validated against the jax reference)."""
    means2D = np.asarray(means2D, np.float32)
    cov2d = np.asarray(cov2d, np.float32)
    color = np.asarray(color, np.float32)
    opacity = np.asarray(opacity, np.float32)
    depths = np.asarray(depths, np.float32)
    c00 = cov2d[:, 0, 0]; c01 = cov2d[:, 0, 1]
    c10 = cov2d[:, 1, 0]; c11 = cov2d[:, 1, 1]
    det = c00 * c11 - c01 * c10
    mid = np.float32(0.5) * (c00 + c11)
    s = np.sqrt(np.maximum(mid * mid - det, np.float32(0.1)))
    radii = np.float32(3.0) * np.ceil(np.sqrt(np.maximum(mid + s, mid - s)))
    rmin = np.clip(means2D - radii[:, None], 0.0, W - 1.0).astype(np.float32)
    rmax = np.clip(means2D + radii[:, None], 0.0, W - 1.0).astype(np.float32)
    inv_det = np.float32(1.0) / det
    conic = np.stack([np.stack([c11, -c01], -1),
                      np.stack([-c10, c00], -1)], -2) * inv_det[:, None, None]
    order = np.argsort(depths, kind="stable")
    a_ = conic[:, 0, 0]; c_ = conic[:, 1, 1]; b_ = conic[:, 0, 1] + conic[:, 1, 0]
    img_c = np.ones((H, W, 3), np.float32)
    img_d = np.zeros((H, W, 1), np.float32)
    img_a = np.zeros((H, W, 1), np.float32)
    for h in range(0, H, TILE):
        for w in range(0, W, TILE):
            tlx = np.maximum(rmin[:, 0], w); tly = np.maximum(rmin[:, 1], h)
            brx = np.minimum(rmax[:, 0], w + TILE - 1.0)
            bry = np.minimum(rmax[:, 1], h + TILE - 1.0)
            mask = (brx > tlx) & (bry > tly)
            sel = order[mask[order]]
            mx = means2D[sel, 0]; my = means2D[sel, 1]
            ys, xs = np.meshgrid(np.arange(TILE), np.arange(TILE), indexing="ij")
            pxa = (xs.ravel() + w).astype(np.float32)
            pya = (ys.ravel() + h).astype(np.float32)
            dx = pxa[:, None] - mx[None, :]
            dy = pya[:, None] - my[None, :]
            q = (np.float32(-0.5)
                 * (dx * dx * a_[sel] + dy * dy * c_[sel] + dx * dy * b_[sel]))
            alpha = np.exp(q, dtype=np.float32) * opacity[sel, 0][None, :]
            alpha = np.minimum(alpha, np.float32(0.99))
            u = (np.float32(1.0) - alpha).astype(np.float32)
            P = np.cumprod(u, axis=1, dtype=np.float32)
            Pexc = np.concatenate([np.ones((1024, 1), np.float32), P[:, :-1]], 1)
            wgt = Pexc * alpha
            acc = wgt.sum(1, dtype=np.float32)
            tc_ = wgt @ color[sel] + (np.float32(1.0) - acc)[:, None]
            td = wgt @ depths[sel]
            img_c[h:h + TILE, w:w + TILE] = tc_.reshape(TILE, TILE, 3)
            img_d[h:h + TILE, w:w + TILE] = td.reshape(TILE, TILE, 1)
            img_a[h:h + TILE, w:w + TILE] = acc.reshape(TILE, TILE, 1)
    return img_c, img_d, img_a


# ----------------------------------------------------------------------------
# device kernel
# ----------------------------------------------------------------------------

def _build_nc(padg):
    import concourse.bass as bass
    import concourse.bacc as bacc
    import concourse.tile as tile
    from concourse import mybir
    from contextlib import ExitStack

    F32 = mybir.dt.float32
    BF16 = mybir.dt.bfloat16
    AF = mybir.ActivationFunctionType
    OP = mybir.AluOpType
    pmax = max(padg)
    nblk = [p // 128 for p in padg]

    nc = bacc.Bacc("TRN2", target_bir_lowering=False, debug=False,
                   num_devices=N_CORES)
    feat_d = nc.declare_dram_parameter("feat", [TILES_PER_CORE, 6, 1024], F32,
                                       isOutput=False)
    gco_d = nc.declare_dram_parameter("gco", [TILES_PER_CORE, 6, pmax], F32,
                                      isOutput=False)
    dft_d = nc.declare_dram_parameter("dft", [TILES_PER_CORE, pmax, 5], F32,
                                      isOutput=False)
    bias_d = nc.declare_dram_parameter("bias", [TILES_PER_CORE, 5, 1], F32,
                                       isOutput=False)
    ident_d = nc.declare_dram_parameter("ident", [128, 128], F32,
                                        isOutput=False)
    out_d = nc.declare_dram_parameter("out", [TILES_PER_CORE, 5, 1024], F32,
                                      isOutput=True)

    with tile.TileContext(nc) as tc, ExitStack() as ctx:
        const_pool = ctx.enter_context(tc.tile_pool(name="const", bufs=1))
        in_pool = ctx.enter_context(tc.tile_pool(name="inp", bufs=2))
        work = ctx.enter_context(tc.tile_pool(name="work", bufs=2))
        ppool = ctx.enter_context(tc.tile_pool(name="pbuf", bufs=17))
        qpool = ctx.enter_context(tc.tile_pool(name="qpsum", bufs=2,
                                               space="PSUM"))
        ptpool = ctx.enter_context(tc.tile_pool(name="ptpsum", bufs=2,
                                                space="PSUM"))
        imgpool = ctx.enter_context(tc.tile_pool(name="imgpsum", bufs=1,
                                                 space="PSUM"))

        ident = const_pool.tile([128, 128], F32)
        nc.sync.dma_start(ident[:], ident_d.ap())
        zeros = const_pool.tile([128, pmax], F32)
        nc.gpsimd.memset(zeros[:], 0.0)

        for t in range(TILES_PER_CORE):
            G = padg[t]
            feat_t = in_pool.tile([6, 1024], F32, tag="feat")
            nc.sync.dma_start(feat_t[:], feat_d.ap()[t])
            gco_t = in_pool.tile([6, G], F32, tag="gco")
            nc.sync.dma_start(gco_t[:], gco_d.ap()[t][:, :G])
            dft_t = in_pool.tile([128, nblk[t] * 5], F32, tag="dft")
            nc.sync.dma_start(
                dft_t[:].rearrange("p (b f) -> p b f", f=5),
                dft_d.ap()[t][:G].rearrange("(b k) f -> k b f", k=128),
            )
            bias_t = in_pool.tile([5, 1], F32, tag="bias")
            nc.sync.dma_start(bias_t[:], bias_d.ap()[t])

            # phase 1: per pixel-block exponent -> alpha -> cumprod
            Ps = []
            for pb in range(8):
                e = work.tile([128, G], F32, tag="e")
                for lo in range(0, G, 512):
                    hi = min(G, lo + 512)
                    qh = qpool.tile([128, hi - lo], F32, tag="q")
                    nc.tensor.matmul(
                        qh[:], lhsT=feat_t[:, pb * 128:(pb + 1) * 128],
                        rhs=gco_t[:, lo:hi], start=True, stop=True,
                    )
                    nc.scalar.activation(e[:, lo:hi], qh[:], AF.Exp)
                # u = 1 - min(e, 0.99)  (on GpSimd: DVE is the busier engine)
                u0 = work.tile([128, G], F32, tag="u0")
                nc.gpsimd.tensor_scalar(u0[:], e[:], 0.99, -1.0, OP.min,
                                        OP.mult)
                u = work.tile([128, G], F32, tag="u")
                nc.gpsimd.tensor_scalar(u[:], u0[:], 1.0, None, OP.add)
                # P = inclusive cumprod along gaussians (fp32 state, bf16 out)
                P = ppool.tile([128, G], F32, tag="P")
                nc.vector.tensor_tensor_scan(P[:], u[:], zeros[:, :G], 1.0,
                                             OP.mult, OP.add)
                Ps.append(P)

            # phase 2: per gaussian block: transpose all pixel blocks,
            # copy to SBUF, one wide matmul pair accumulating img^T
            img = imgpool.tile([5, 1024], F32)
            for b in range(nblk[t]):
                pt = ptpool.tile([128, 1024], F32)
                for pb in range(8):
                    nc.tensor.transpose(pt[:, pb * 128:(pb + 1) * 128],
                                        Ps[pb][:, b * 128:(b + 1) * 128],
                                        ident[:])
                pts = work.tile([128, 1024], F32, tag="pts")
                if b % 2 == 0:
                    nc.scalar.activation(pts[:], pt[:], AF.Copy)
                else:
                    nc.vector.tensor_copy(pts[:], pt[:])
                for half in range(2):
                    nc.tensor.matmul(
                        img[:, half * 512:(half + 1) * 512],
                        lhsT=dft_t[:, b * 5:(b + 1) * 5],
                        rhs=pts[:, half * 512:(half + 1) * 512],
                        start=(b == 0), stop=(b == nblk[t] - 1),
                    )
            outsb = work.tile([5, 1024], F32, tag="outsb")
            nc.scalar.activation(outsb[:], img[:], AF.Identity,
                                 bias=bias_t[:])
            nc.sync.dma_start(out_d.ap()[t], outsb[:])
    nc.compile()
    return nc


def _get_nc(padg):
    key = ("nc", tuple(padg))
    if key not in _CACHE:
        _CACHE[key] = _build_nc(padg)
    return _CACHE[key]


def _make_in_maps(prep):
    feat, gco, dft, bias, _padg = prep
    ident = np.eye(128, dtype=np.float32)
    in_maps = []
    for c in range(N_CORES):
        sl = slice(c * TILES_PER_CORE, (c + 1) * TILES_PER_CORE)
        in_maps.append({
            "feat": np.ascontiguousarray(feat[sl]),
            "gco": np.ascontiguousarray(gco[sl]),
            "dft": np.ascontiguousarray(dft[sl]),
            "bias": np.ascontiguousarray(bias[sl])[..., None],
            "ident": ident,
        })
    return in_maps


def _gather(results):
    img_c = np.empty((H, W, 3), np.float32)
    img_d = np.empty((H, W, 1), np.float32)
    img_a = np.empty((H, W, 1), np.float32)
    t = 0
    for h in range(0, H, TILE):
        for w in range(0, W, TILE):
            core, slot = divmod(t, TILES_PER_CORE)
            o = results[core]["out"][slot]            # [5, 1024]
            img_c[h:h + TILE, w:w + TILE] = (
                o[0:3].T.reshape(TILE, TILE, 3))
            img_d[h:h + TILE, w:w + TILE] = o[3].reshape(TILE, TILE, 1)
            img_a[h:h + TILE, w:w + TILE] = o[4].reshape(TILE, TILE, 1)
            t += 1
    return img_c, img_d, img_a


def kernel(means2D, cov2d, color, opacity, depths):
    prep = _prep(means2D, cov2d, color, opacity, depths)
    if prep is None:
        return _render_numpy(means2D, cov2d, color, opacity, depths)
    from concourse.bass_utils import run_bass_kernel_spmd
    nc = _get_nc(prep[4])
    in_maps = _make_in_maps(prep)
    res = run_bass_kernel_spmd(nc, in_maps, core_ids=list(range(N_CORES)))
    return _gather(res.results)
